# revision 11
# baseline (speedup 1.0000x reference)
"""Trainium2 Bass kernel for nn_DescriptionAware (dense_mlp).

Self-contained: takes FULL inputs (as in reference.setup_inputs()), shards
across 8 NeuronCores (batch x class-half), runs one SPMD Bass/Tile program,
reassembles the full [B,S,C] f32 logits on host.

Sharding: core k handles batch b=k//2 and classes [32*(k%2), 32*(k%2)+32).

Gathers use mainline SWDGE indirect DMA ([128,1] int32 offset tables, one
128-row gather per instruction, ~1.1us serial each on GpSimd). Slot layout:
rank 2+c gathers class c's 8 senses x 16 tokens (partition p = 16n+l), so
the per-octet compute pipeline chases the gather stream class by class.
Word table is bf16 [V,300]; weights bf16; final W2 contraction merges the
44-row tail chunk across class pairs to save PE streams.
"""

import os
import numpy as np

import concourse.bass as bass
import concourse.mybir as mybir
import concourse.tile as tile
from concourse import bacc
from concourse.bass import IndirectOffsetOnAxis
from concourse.bass_utils import run_bass_kernel_spmd
from concourse.tile_rust import add_dep_helper

# problem dims (hardcoded per contract)
B, S, H = 4, 256, 768
C = 64
LD = 128
E = 300
NS = 8
LP = 32
LA = 16
V = 50000
DH = 300

NCORES = 8
CH = 32                      # classes per core
DCH = [(0, 128), (128, 256), (256, 300)]   # d-chunks of DH=300
HCH = 6                      # 768 = 6*128
KL = [(0, 128), (128, 256), (256, 384), (384, 428)]  # W1l row chunks (rel)
KA = [(128 * i, min(128 * (i + 1), 1069)) for i in range(9)]     # Wa1_aug row chunks

R_PD = 2                     # pd ranks (rank 0: senses 0-3, rank 1: senses 4-7)
RANKS = R_PD + CH            # 34: rank 2+c gathers class c (p = 16n+l)

F32 = mybir.dt.float32
F32R = mybir.dt.float32r
BF16 = mybir.dt.bfloat16
I32 = mybir.dt.int32
AL = mybir.AluOpType
AF = mybir.ActivationFunctionType

# c128 const column layout (f32r)
ID_0 = 0           # 128 cols identity
ONES_0 = 128       # 128 cols ones
P16_0 = 256        # 8 cols: delta(p//16 == n), transposed use
NC128 = 264
# c8 const column layout (f32r)
ONEC_0 = 0         # 1 col ones
ONEW_0 = 1         # 8 cols ones (row 0 used as [1,8] ones)
P16T_0 = 9         # 128 cols: delta(n == p//16) as [8, 128]
NC8 = 137


def _host_consts():
    c128 = np.zeros((128, NC128), np.float32)
    c128[:, ID_0:ID_0 + 128] = np.eye(128, dtype=np.float32)
    c128[:, ONES_0:ONES_0 + 128] = 1.0
    p = np.arange(128)
    for n in range(8):
        c128[p // 16 == n, P16_0 + n] = 1.0
    c8 = np.zeros((8, NC8), np.float32)
    c8[:, ONEC_0] = 1.0
    c8[:, ONEW_0:ONEW_0 + 8] = 1.0
    n_ = np.arange(8)
    c8[:, P16T_0:P16T_0 + 128] = (n_[:, None] == (np.arange(128)[None, :] // 16)).astype(np.float32)
    return c128, c8


def build_program():
    nc = bacc.Bacc("TRN2", target_bir_lowering=False, debug=False, num_devices=NCORES,
                   dynamic_dma_scratch_size=65536)

    dt = nc.dram_tensor
    t_tab = dt("tab", [V, E], BF16, kind="ExternalInput")
    t_idx = dt("idx", [128, RANKS], I32, kind="ExternalInput")
    t_hostC = dt("hostC", [128, CH * NS], BF16, kind="ExternalInput")
    t_pdsel = dt("pdsel", [128, R_PD * NS], BF16, kind="ExternalInput")
    t_xT = dt("xT", [128, HCH * S], BF16, kind="ExternalInput")
    t_smrow = dt("smrow", [128, HCH * 128], F32, kind="ExternalInput")
    t_sbias = dt("sbias", [NS, 1], F32, kind="ExternalInput")
    t_lembT = dt("lembT", [LD, CH], BF16, kind="ExternalInput")
    t_wa1 = dt("wa1", [128, 9 * H], BF16, kind="ExternalInput")
    t_wa2b = dt("wa2b", [NS, H], F32, kind="ExternalInput")
    t_w1x = dt("w1x", [128, HCH * DH], BF16, kind="ExternalInput")
    t_w1x3 = dt("w1x3", [128, HCH * 108], BF16, kind="ExternalInput")
    t_w1l = dt("w1l", [128, 4 * 364], BF16, kind="ExternalInput")
    t_w1p = dt("w1p", [128, HCH * 364], BF16, kind="ExternalInput")
    t_w2 = dt("w2", [128, 2 * 32], BF16, kind="ExternalInput")
    t_w23p = dt("w23p", [108, 64], BF16, kind="ExternalInput")
    t_b1r = dt("b1r", [1, 364], F32, kind="ExternalInput")
    t_b2b = dt("b2b", [128, 1], F32, kind="ExternalInput")
    t_c128 = dt("c128", [128, NC128], F32R, kind="ExternalInput")
    t_c8 = dt("c8", [8, NC8], F32R, kind="ExternalInput")
    t_out = dt("out", [16, 512], F32, kind="ExternalOutput")

    with tile.TileContext(nc) as tc:
        with tc.tile_pool(name="sb", bufs=1) as sb, \
             tc.tile_pool(name="sbt", bufs=6) as sbt, \
             tc.tile_pool(name="ppw", bufs=2, space="PSUM") as ppw, \
             tc.tile_pool(name="ppa", bufs=2, space="PSUM") as ppa, \
             tc.tile_pool(name="pph", bufs=2, space="PSUM") as pph, \
             tc.tile_pool(name="ppo", bufs=1, space="PSUM") as ppo:

            # ---------------- DMAs (order = per-engine issue order) ----------------
            # sync (HWDGE) queue: gather offsets first, then attention/x weights
            idxt = sb.tile([128, RANKS], I32, tag="idxt")
            nc.sync.dma_start(out=idxt[:], in_=t_idx[:])
            c8 = sb.tile([8, NC8], F32R, tag="c8")
            nc.sync.dma_start(out=c8[:], in_=t_c8[:])
            c128 = sb.tile([128, NC128], F32R, tag="c128")
            nc.sync.dma_start(out=c128[:], in_=t_c128[:])
            pdsel = sb.tile([128, R_PD * NS], BF16, tag="pdsel")
            nc.sync.dma_start(out=pdsel[:], in_=t_pdsel[:])
            xT = sb.tile([128, HCH * S], BF16, tag="xT")
            nc.sync.dma_start(out=xT[:], in_=t_xT[:])
            wa1_all = sb.tile([128, 9 * H], BF16, tag="wa1_all")
            nc.sync.dma_start(out=wa1_all[:], in_=t_wa1[:])
            wa1 = [wa1_all[0:(r1 - r0), H * i:H * (i + 1)] for i, (r0, r1) in enumerate(KA)]
            w1x_all = sb.tile([128, HCH * DH], BF16, tag="w1x_all")
            nc.sync.dma_start(out=w1x_all[:], in_=t_w1x[:])
            w1x = [w1x_all[:, DH * hc:DH * (hc + 1)] for hc in range(HCH)]
            w1x3_all = sb.tile([128, HCH * 108], BF16, tag="w1x3_all")
            nc.sync.dma_start(out=w1x3_all[:], in_=t_w1x3[:])
            w1x3 = [w1x3_all[:, 108 * hc:108 * (hc + 1)] for hc in range(HCH)]
            hostC = sb.tile([128, CH * NS], BF16, tag="hostC")
            nc.sync.dma_start(out=hostC[:], in_=t_hostC[:])

            # gpsimd queue: ONLY the 34 serial indirect gathers (pd ranks first)
            G = sb.tile([128, RANKS, E], BF16, tag="G")
            for r in range(RANKS):
                nc.gpsimd.indirect_dma_start(
                    out=G[:, r, :], out_offset=None, in_=t_tab[:],
                    in_offset=IndirectOffsetOnAxis(ap=idxt[:, r:r + 1], axis=0))

            # scalar (HWDGE) queue: the rest
            smrow = sb.tile([128, HCH * 128], F32, tag="smrow")
            nc.scalar.dma_start(out=smrow[:], in_=t_smrow[:])
            sbias = sb.tile([NS, 1], F32, tag="sbias")
            nc.scalar.dma_start(out=sbias[:], in_=t_sbias[:])
            wa2b = sb.tile([NS, H], F32, tag="wa2b")
            nc.scalar.dma_start(out=wa2b[:], in_=t_wa2b[:])
            b2b = sb.tile([128, 1], F32, tag="b2b")
            nc.scalar.dma_start(out=b2b[:], in_=t_b2b[:])
            b1r = sb.tile([1, 364], F32, tag="b1r")
            nc.scalar.dma_start(out=b1r[:], in_=t_b1r[:])
            w1p_all = sb.tile([128, HCH * 364], BF16, tag="w1p_all")
            nc.scalar.dma_start(out=w1p_all[:], in_=t_w1p[:])
            w1p = [w1p_all[:, 364 * i:364 * (i + 1)] for i in range(HCH)]
            w1l_all = sb.tile([128, 4 * 364], BF16, tag="w1l_all")
            nc.scalar.dma_start(out=w1l_all[:], in_=t_w1l[:])
            w1l = [w1l_all[0:(r1 - r0), 364 * i:364 * (i + 1)] for i, (r0, r1) in enumerate(KL)]
            lembT = sb.tile([LD, CH], BF16, tag="lembT")
            nc.scalar.dma_start(out=lembT[:], in_=t_lembT[:])
            w2_all = sb.tile([128, 2 * 32], BF16, tag="w2_all")
            nc.scalar.dma_start(out=w2_all[:], in_=t_w2[:])
            w2c = [w2_all[0:128, 32 * i:32 * (i + 1)] for i in range(2)]
            w23p = sb.tile([108, 64], BF16, tag="w23p")
            nc.scalar.dma_start(out=w23p[:], in_=t_w23p[:])

            ident = c128[:, ID_0:ID_0 + 128]
            ones128row = c128[0:1, ONES_0:ONES_0 + 128].bitcast(F32)
            ones8row = c8[0:1, ONEW_0:ONEW_0 + 8].bitcast(F32)

            # ---------------- pred span mean-pool (vector, from xT) ----------------
            predT6 = sb.tile([128, HCH], F32, tag="predT6")
            for hc in range(HCH):
                pm = sbt.tile([128, 128], F32, tag="pm")
                nc.vector.tensor_tensor(
                    out=pm[:], in0=xT[:, S * hc:S * hc + 128],
                    in1=smrow[:, 128 * hc:128 * (hc + 1)], op=AL.mult)
                nc.vector.tensor_reduce(
                    out=predT6[:, hc:hc + 1], in_=pm[:],
                    axis=mybir.AxisListType.X, op=AL.add)
            predT6b = sb.tile([128, HCH], BF16, tag="predT6b")
            nc.vector.tensor_copy(out=predT6b[:], in_=predT6[:])

            # ---------------- pd_agg from pd ranks 0,1 ----------------
            pdps = ppw.tile([NS, E], F32, tag="w", name="pdps")
            for r in range(R_PD):
                nc.tensor.matmul(out=pdps[:], lhsT=pdsel[:, NS * r:NS * (r + 1)],
                                 rhs=G[:, r, :],
                                 start=(r == 0), stop=(r == R_PD - 1))
            pd_agg = sb.tile([NS, E], F32R, tag="pd_agg")
            nc.vector.tensor_copy(out=pd_agg[:], in_=pdps[:])

            # ---------------- attention MLP -> per-slot sense weight wx ----------------
            attk = []
            for kk in range(HCH):
                a_ = sb.tile([128, 8], BF16, tag=f"attk{kk}", name=f"attk{kk}")
                nc.vector.tensor_copy(out=a_[:], in_=predT6b[:, kk:kk + 1].to_broadcast([128, 8]))
                attk.append(a_)
            for e in range(2):
                tp = ppw.tile([128, 8], F32R, tag="w", name=f"tpa{e}")
                nc.tensor.transpose(out=tp[:], in_=pd_agg[:, 128 * e:128 * (e + 1)],
                                    identity=ident[0:8, 0:8])
                a_ = sb.tile([128, 8], BF16, tag=f"attk{HCH + e}", name=f"attk{HCH + e}")
                nc.vector.tensor_copy(out=a_[:], in_=tp[:].bitcast(F32))
                attk.append(a_)
            tp = ppw.tile([44, 8], F32R, tag="w", name="tpb")
            nc.tensor.transpose(out=tp[:], in_=pd_agg[:, 256:300], identity=ident[0:8, 0:8])
            a_ = sb.tile([45, 8], BF16, tag="attk8")
            nc.vector.memset(a_[:, :], 1.0)
            nc.vector.tensor_copy(out=a_[0:44, :], in_=tp[:].bitcast(F32))
            attk.append(a_)

            hidp = []
            for nb in range(2):
                hp2 = ppw.tile([8, 384], F32, tag="w", name=f"hid{nb}")
                for kk in range(9):
                    nc.tensor.matmul(out=hp2[:], lhsT=attk[kk][:],
                                     rhs=wa1[kk][:, 384 * nb:384 * (nb + 1)],
                                     start=(kk == 0), stop=(kk == 8))
                hidp.append(hp2)
            hid = sb.tile([8, H], F32, tag="hid")
            for nb in range(2):
                nc.scalar.activation(out=hid[:, 384 * nb:384 * (nb + 1)], in_=hidp[nb][:],
                                     func=AF.Relu)
            scr = sb.tile([8, H], F32, tag="scr")
            nc.vector.tensor_tensor(out=scr[:], in0=hid[:], in1=wa2b[:], op=AL.mult)
            wraw = sb.tile([8, 1], F32, tag="wraw")
            nc.vector.tensor_reduce(out=wraw[:], in_=scr[:], axis=mybir.AxisListType.X,
                                    op=AL.add)
            wsb = sb.tile([8, 1], F32, tag="wsb")
            nc.vector.tensor_scalar(out=wsb[:], in0=wraw[:], scalar1=sbias[:],
                                    scalar2=None, op0=AL.add)
            expc = sb.tile([8, 1], F32R, tag="expc")
            nc.scalar.activation(out=expc[:], in_=wsb[:], func=AF.Exp)
            sps = ppw.tile([1, 1], F32, tag="w", name="sps")
            nc.tensor.matmul(out=sps[:], lhsT=expc[:].bitcast(F32),
                             rhs=c8[:, ONEC_0:ONEC_0 + 1].bitcast(F32),
                             start=True, stop=True)
            rs = sb.tile([1, 1], F32, tag="rs")
            nc.vector.reciprocal(out=rs[:], in_=sps[:])
            rbps = ppw.tile([8, 1], F32, tag="w", name="rbps")
            nc.tensor.matmul(out=rbps[:], lhsT=ones8row, rhs=rs[:], start=True, stop=True)
            wcol = sb.tile([8, 1], F32R, tag="wcol")
            nc.vector.tensor_tensor(out=wcol[:], in0=expc[:].bitcast(F32), in1=rbps[:],
                                    op=AL.mult)
            # wx[p] = weights[p//16] (slot layout: p = 16n+l)
            wxps = ppw.tile([128, 1], F32, tag="w", name="wxps")
            nc.tensor.matmul(out=wxps[:], lhsT=c8[:, P16T_0:P16T_0 + 128].bitcast(F32),
                             rhs=wcol[:].bitcast(F32), start=True, stop=True)
            wx = sb.tile([128, 1], F32, tag="wx")
            nc.vector.tensor_copy(out=wx[:], in_=wxps[:])

            # ---------------- hxT (chunk 2 duplicated to 88 rows) ----------------
            hxT = []
            for dc in range(3):
                ds_ = 128 if dc < 2 else 108
                hp_ = ppw.tile([ds_, S], F32, tag="w", name=f"hp_{dc}")
                for hc in range(HCH):
                    lh = w1x[hc][:, 128 * dc:128 * (dc + 1)] if dc < 2 else w1x3[hc][:]
                    nc.tensor.matmul(out=hp_[:], lhsT=lh, rhs=xT[:, S * hc:S * (hc + 1)],
                                     start=(hc == 0), stop=(hc == HCH - 1))
                hs = sb.tile([ds_, S], F32, tag=f"hxT{dc}")
                nc.scalar.copy(out=hs[:], in_=hp_[:])
                hxT.append(hs)

            # ---------------- hp row -> hpbT (chunk 2 duplicated) ----------------
            hprow = ppw.tile([1, 364], F32, tag="w", name="hprow")
            for i in range(HCH):
                nc.tensor.matmul(out=hprow[:], lhsT=predT6b[:, i:i + 1], rhs=w1p[i][:],
                                 start=(i == 0), stop=(i == HCH - 1), tile_position=(0, 0))
            hpb = sb.tile([1, 364], F32R, tag="hpb")
            nc.vector.tensor_tensor(out=hpb[:], in0=hprow[:], in1=b1r[:], op=AL.add)
            hpbT = []
            for dc in range(2):
                tp2 = ppw.tile([128, 1], F32R, tag="w", name=f"tp2{dc}")
                nc.tensor.transpose(out=tp2[:].bitcast(F32),
                                    in_=hpb[0:1, 128 * dc:128 * (dc + 1)].bitcast(F32),
                                    identity=ident[0:1, 0:1].bitcast(F32))
                hb = sb.tile([128, 1], F32, tag=f"hpbT{dc}")
                nc.vector.tensor_copy(out=hb[:], in_=tp2[:].bitcast(F32))
                hpbT.append(hb)
            tp2d = ppw.tile([108, 1], F32R, tag="w", name="tp2d")
            nc.tensor.transpose(out=tp2d[:].bitcast(F32),
                                in_=hpb[0:1, 256:364].bitcast(F32),
                                identity=ident[0:1, 0:1].bitcast(F32))
            hb3 = sb.tile([108, 1], F32, tag="hpbT2")
            nc.vector.tensor_copy(out=hb3[:], in_=tp2d[:].bitcast(F32))
            hpbT.append(hb3)

            # ---------------- per-octet pipeline ----------------
            prev_end = [None, None]
            outp2 = None
            for oc in range(4):
                cyc = oc // 2
                if oc % 2 == 0:
                    outp2 = [ppo.tile([128, 512], F32, tag=f"out{h}", name=f"outp{h}_{cyc}")
                             for h in range(2)]
                    prev_end = [None, None]
                rowbase = 64 * (oc % 2)

                # arg weighted sums for classes 8oc..8oc+7
                aw = ppa.tile([8, E], F32, tag="aw", name=f"aw{oc}")
                for j in range(8):
                    c = 8 * oc + j
                    wsel = sbt.tile([128, 8], BF16, tag="wsel")
                    nc.vector.tensor_scalar(out=wsel[:], in0=hostC[:, 8 * c:8 * (c + 1)],
                                            scalar1=wx[:], scalar2=None, op0=AL.mult)
                    nc.tensor.matmul(out=aw[:], lhsT=wsel[:], rhs=G[:, R_PD + c, :],
                                     start=(j == 0), stop=(j == 7))
                aws = sbt.tile([8, E], F32R, tag="aws")
                nc.vector.tensor_copy(out=aws[:], in_=aw[:])
                awT = []
                for e, (e0, e1) in enumerate(DCH):
                    tp3 = ppw.tile([e1 - e0, 8], F32R, tag="w", name=f"tp3{e}")
                    nc.tensor.transpose(out=tp3[:], in_=aws[:, e0:e1],
                                        identity=ident[0:8, 0:8])
                    li = sbt.tile([e1 - e0, 8], BF16, tag=f"awT{e}")
                    nc.vector.tensor_copy(out=li[:], in_=tp3[:].bitcast(F32))
                    awT.append(li)

                # hl for this octet
                hl = pph.tile([8, 364], F32, tag="hl", name=f"hl{oc}")
                for kc in range(4):
                    lh = lembT[:, 8 * oc:8 * (oc + 1)] if kc == 0 else awT[kc - 1][:]
                    nc.tensor.matmul(out=hl[:], lhsT=lh, rhs=w1l[kc][:],
                                     start=(kc == 0), stop=(kc == 3))
                hls = sbt.tile([8, 364], F32R, tag="hls")
                nc.vector.tensor_copy(out=hls[:], in_=hl[:])

                biasT = []
                for dc in range(2):
                    tp4 = ppw.tile([128, 8], F32R, tag="w", name=f"tp4{dc}")
                    nc.tensor.transpose(out=tp4[:], in_=hls[:, 128 * dc:128 * (dc + 1)],
                                        identity=ident[0:8, 0:8])
                    bt = sbt.tile([128, 8], F32, tag=f"biasT{dc}", name=f"bt{dc}")
                    nc.vector.tensor_scalar(out=bt[:], in0=tp4[:].bitcast(F32),
                                            scalar1=hpbT[dc][:], scalar2=None, op0=AL.add)
                    biasT.append(bt)
                tp4d = ppw.tile([108, 8], F32R, tag="w", name="tp4d")
                nc.tensor.transpose(out=tp4d[:], in_=hls[:, 256:364],
                                    identity=ident[0:8, 0:8])
                bt3 = sbt.tile([108, 8], F32, tag="biasT2", name="bt3")
                nc.vector.tensor_scalar(out=bt3[:], in0=tp4d[:].bitcast(F32),
                                        scalar1=hpbT[2][:], scalar2=None, op0=AL.add)
                biasT.append(bt3)

                # final: 4 pairs (d-chunks 0,1) + 2 merged chunk-2 matmuls
                for cl in range(4):
                    cp = 4 * oc + cl
                    h = cp % 2
                    row = rowbase + 32 * (cl // 2)
                    for dc in range(2):
                        tt = sbt.tile([128, 512], BF16, tag="t", name="tt")
                        nc.scalar.activation(out=tt[:, 0:256], in_=hxT[dc][:], func=AF.Relu,
                                             bias=biasT[dc][:, 2 * cl:2 * cl + 1])
                        nc.vector.tensor_scalar(out=tt[:, 256:512], in0=hxT[dc][:],
                                                scalar1=biasT[dc][:, 2 * cl + 1:2 * cl + 2],
                                                scalar2=0.0, op0=AL.add, op1=AL.max)
                        mm = nc.tensor.matmul(out=outp2[h][row:row + 32, :], lhsT=w2c[dc][:],
                                              rhs=tt[:], start=(dc == 0), stop=False,
                                              tile_position=(0, row), skip_group_check=True)
                        if dc == 0 and prev_end[h] is not None:
                            add_dep_helper(mm.ins, prev_end[h], sync=False,
                                           reason="serialize psum groups per bank")
                # merged chunk-2 per h: pairs (4oc+h, 4oc+2+h) at rows rowbase, rowbase+32
                for h in range(2):
                    tt3 = sbt.tile([108, 512], BF16, tag="t3", name="tt3")
                    for half in range(2):
                        cl = 2 * half + h
                        p0, p1 = (0, 64) if half == 0 else (64, 108)
                        nc.scalar.activation(out=tt3[p0:p1, 0:256], in_=hxT[2][p0:p1, :],
                                             func=AF.Relu,
                                             bias=bt3[p0:p1, 2 * cl:2 * cl + 1])
                        nc.vector.tensor_scalar(out=tt3[p0:p1, 256:512],
                                                in0=hxT[2][p0:p1, :],
                                                scalar1=bt3[p0:p1, 2 * cl + 1:2 * cl + 2],
                                                scalar2=0.0, op0=AL.add, op1=AL.max)
                    mm = nc.tensor.matmul(out=outp2[h][rowbase:rowbase + 64, :],
                                          lhsT=w23p[:], rhs=tt3[:], start=False, stop=True,
                                          tile_position=(0, rowbase), skip_group_check=True)
                    prev_end[h] = mm.ins

                if oc % 2 == 1:
                    for h2 in range(2):
                        osb = sb.tile([128, 512], F32, tag=f"osb{cyc}{h2}",
                                      name=f"osb{cyc}{h2}")
                        nc.vector.tensor_scalar(out=osb[:], in0=outp2[h2][:],
                                                scalar1=b2b[:, :], scalar2=None, op0=AL.add)
                        nc.sync.dma_start(out=t_out[8 * cyc + h2:8 * cyc + 8:2, :],
                                          in_=osb[0:128:32, :])

    nc.compile()
    return nc


def _pack(a, rows, cols):
    # [k*128, cols] -> [128, k*cols] p-major
    k = rows // 128
    return np.ascontiguousarray(a.reshape(k, 128, cols).transpose(1, 0, 2).reshape(128, k * cols))


def make_in_maps(inputs):
    import ml_dtypes
    pack = _pack
    x = np.asarray(inputs["x"], np.float32)
    pred_start = np.asarray(inputs["pred_start"]).astype(np.int64)
    pred_end = np.asarray(inputs["pred_end"]).astype(np.int64)
    pdi = np.asarray(inputs["pred_desc_ids"]).astype(np.int64)
    adi = np.asarray(inputs["arg_desc_ids"]).astype(np.int64)
    label_emb = np.asarray(inputs["label_emb"], np.float32)
    word_emb = np.asarray(inputs["word_emb"], np.float32)
    Wa1 = np.asarray(inputs["Wa1"], np.float32)
    ba1 = np.asarray(inputs["ba1"], np.float32)
    Wa2 = np.asarray(inputs["Wa2"], np.float32)
    ba2 = np.asarray(inputs["ba2"], np.float32)
    W1 = np.ascontiguousarray(np.asarray(inputs["W1"], np.float32))
    b1 = np.asarray(inputs["b1"], np.float32)
    W2c = np.asarray(inputs["W2"], np.float32).reshape(DH)
    b2 = np.asarray(inputs["b2"], np.float32)

    tab = np.ascontiguousarray(word_emb).astype(ml_dtypes.bfloat16)

    # W2 chunk lhsTs: chunks 0,1 (col 0 of [128,32] blocks); chunk2 merged [88,64]
    W2p = np.zeros((256, 32), np.float32)
    W2p[0:128, 0] = W2c[0:128]
    W2p[128:256, 0] = W2c[128:256]
    W2_p = pack(W2p, 256, 32).astype(ml_dtypes.bfloat16)
    W23p = np.zeros((108, 64), np.float32)
    W23p[0:44, 0] = W2c[256:300]
    W23p[64:108, 32] = W2c[256:300]
    W23p = W23p.astype(ml_dtypes.bfloat16)

    c128, c8 = _host_consts()
    wa1_aug = np.zeros((1152, H), np.float32)
    wa1_aug[:1068] = Wa1
    wa1_aug[1068] = ba1
    wa1_p = pack(wa1_aug, 1152, H).astype(ml_dtypes.bfloat16)
    w1x_p = pack(W1[0:768], 768, DH).astype(ml_dtypes.bfloat16)
    def _ext364(w):
        # [k, 300] -> [k, 364]: cols 300:320 zero, 320:364 = chunk3 dup
        out = np.zeros((w.shape[0], 364), np.float32)
        out[:, 0:300] = w
        out[:, 320:364] = w[:, 256:300]
        return out

    w1x3 = np.zeros((768, 108), np.float32)
    w1x3[:, 0:44] = W1[0:768, 256:300]
    w1x3[:, 64:108] = W1[0:768, 256:300]
    w1x3_p = pack(np.ascontiguousarray(w1x3), 768, 108).astype(ml_dtypes.bfloat16)
    w1l_p = pack(_ext364(W1[768:1280]), 512, 364).astype(ml_dtypes.bfloat16)
    w1p_p = pack(_ext364(W1[1196:1964]), 768, 364).astype(ml_dtypes.bfloat16)
    wa2b_v = np.ascontiguousarray(np.tile(Wa2.reshape(1, H), (NS, 1)))
    b2b_v = np.full((128, 1), float(b2[0]), np.float32)
    b1row = np.ascontiguousarray(_ext364(b1.reshape(1, DH)))

    in_maps = []
    for core in range(NCORES):
        b, ch = core // 2, core % 2

        ids = adi[b, :, ch * CH:(ch + 1) * CH, :]          # [8, 32, 16] (n, c, l)
        m = (ids > 0)
        ln = np.maximum(1, m.sum(-1)).astype(np.float32)   # [8, 32]
        pids = pdi[b]                                      # [8, 32] (n, l)
        pm = (pids > 0)
        pln = np.maximum(1, pm.sum(-1)).astype(np.float32)  # [8]
        sbias_v = (np.where(pm.sum(-1) > 0, 0.0, -100000.0)
                   + float(ba2[0])).astype(np.float32).reshape(NS, 1)

        # offsets: rank 0,1 = pd (p = 32n' + l, 4 senses per rank);
        # rank 2+c = class c args (p = 16n + l)
        idxm = np.zeros((128, RANKS), np.int32)
        idxm[:, 0] = pids[0:4].reshape(-1)
        idxm[:, 1] = pids[4:8].reshape(-1)
        idxm[:, R_PD:] = ids.transpose(1, 0, 2).reshape(CH, 128).T

        # pdsel[p, 8r+n] = delta(sense(p,r)==n) * mask/plen
        pS = np.zeros((128, R_PD, NS), np.float32)
        for r in range(R_PD):
            for n4 in range(4):
                n = 4 * r + n4
                rows = slice(32 * n4, 32 * (n4 + 1))
                pS[rows, r, n] = pm[n] / pln[n]
        # hostC[p, 8c+j] = delta(j == c%8) * mask/len  (p = 16n+l)
        hC = np.zeros((128, CH, NS), np.float32)
        for cc in range(CH):
            hC[:, cc, cc % 8] = (m[:, cc, :] / ln[:, cc][:, None]).reshape(128)

        ps_, pe_ = int(pred_start[b]), int(pred_end[b])
        sm = np.zeros(128, np.float32)
        sm[ps_:min(pe_, 128)] = 1.0 / max(1, pe_ - ps_)
        smrow_v = np.ascontiguousarray(
            np.tile(np.tile(sm, HCH).reshape(1, HCH * 128), (128, 1)))

        in_maps.append({
            "tab": tab,
            "idx": idxm,
            "hostC": np.ascontiguousarray(hC.reshape(128, CH * NS)).astype(ml_dtypes.bfloat16),
            "pdsel": np.ascontiguousarray(pS.reshape(128, R_PD * NS)).astype(ml_dtypes.bfloat16),
            "xT": pack(np.ascontiguousarray(x[b].T), H, S).astype(ml_dtypes.bfloat16),
            "smrow": smrow_v,
            "sbias": sbias_v,
            "lembT": np.ascontiguousarray(
                label_emb[ch * CH:(ch + 1) * CH, :].T).astype(ml_dtypes.bfloat16),
            "wa1": wa1_p, "wa2b": wa2b_v, "b2b": b2b_v,
            "w1x": w1x_p, "w1x3": w1x3_p, "w1l": w1l_p, "w1p": w1p_p,
            "w2": W2_p, "w23p": W23p,
            "b1r": b1row,
            "c128": c128, "c8": c8,
        })
    return in_maps


def assemble(results):
    logits = np.empty((B, S, C), np.float32)
    for core in range(NCORES):
        b, ch = core // 2, core % 2
        r = results[core]["out"].reshape(CH, S)
        logits[b, :, ch * CH:(ch + 1) * CH] = r.T
    return logits


_NC_CACHE = {}
LAST_RESULTS = None


def kernel(**inputs):
    global LAST_RESULTS
    if "nc" not in _NC_CACHE:
        _NC_CACHE["nc"] = build_program()
    nc = _NC_CACHE["nc"]
    in_maps = make_in_maps(inputs)
    trace = bool(os.environ.get("KBENCH_TRACE"))
    res = run_bass_kernel_spmd(nc, in_maps, core_ids=list(range(NCORES)), trace=trace)
    LAST_RESULTS = res
    return assemble(res.results)
